# revision 31
# baseline (speedup 1.0000x reference)
"""Trainium2 Bass kernel for BasicPGCBlock:
   per-pixel Gaussian smoothing (5x5, sigma = cubic(perspective)) -> dilated 3x3 conv (256->256) + bias + ReLU.

Sharding: data-parallel over batch, 1 image per NeuronCore (8 cores).

Math: the per-pixel 5x5 kernel w(u,v) = exp(-(u^2+v^2)/(2 s^2)) / Z factors through
t = exp(-1/(2 s^2)):  w(u,v) = t^(u^2+v^2) / Z, and u^2+v^2 in {0,1,2,4,5,8}.
So smoothed = sum_m c_m * S_m with c_m = t^m / Z (host-computed per-pixel planes,
replicated across partitions) and S_m = fixed 0/1 stencil sums of x built from
shifted adds.

Engine balance (cost-model-driven):
  PE:   dilated conv (18 accumulating matmuls per 4-row chunk per half-out) +
        S5 built via identity matmuls (4 taps accumulated in PSUM).
  DVE:  P1/P2 horizontal sums, S1/S4 vertical builds, the c0/c1/c4/c5 apply
        chain and final merges (2x fp16 mode).
  Pool: S2/S8 vertical builds and their c2/c8 multiplies (otherwise idle).
  Act:  S5 PSUM evacuation, conv bias+ReLU evacuation.

All smoothing in fp16 (same DVE/PE cost as bf16, 4x less rounding error),
conv in fp16 with f32 PSUM accumulation.
"""

import sys

sys.path.insert(0, "/opt/trn_rl_repo")

import numpy as np
import ml_dtypes

B, C, H, W = 8, 256, 96, 96
HP, WP = H + 4, W + 4          # zero-padded by 2 on each side
CHUNK = 4                      # conv output rows per matmul (N = 4*96 = 384 <= 512)
OFFS = (-2, 0, 2)              # dilated conv offsets
MS = (0, 1, 2, 4, 5, 8)        # exponents of t present in the 5x5 kernel

_cache = {}


def _slabs(n=8):
    assert H % n == 0
    return tuple((r, n) for r in range(0, H, n))


def _build(repeats=1, loop=None, fp16=False, pool_ops=0, pe_s5=True,
           pe_s2=True, pe_s8=False, slabs=None, unroll=1, flush_min=3,
           flush_min_first=3, s5_dve_tail=0):
    import concourse.mybir as mybir
    from concourse import bacc
    from concourse.tile import TileContext

    dt = mybir.dt
    DT = dt.float16 if fp16 else dt.bfloat16
    SLABS = slabs if slabs is not None else _slabs()
    nc = bacc.Bacc("TRN2", target_bir_lowering=False, debug=False)

    xp = nc.dram_tensor("xp", (128, 2, HP, WP), DT, kind="ExternalInput").ap()
    cpl = nc.dram_tensor("cpl", (128, 6, H, W), DT, kind="ExternalInput").ap()
    wts = nc.dram_tensor("wts", (2, 128, 9 * 2 * 128), DT, kind="ExternalInput").ap()
    bias = nc.dram_tensor("bias", (128, 2), dt.float32, kind="ExternalInput").ap()
    ident = nc.dram_tensor("ident", (128, 128), DT, kind="ExternalInput").ap()
    y = nc.dram_tensor("y", (2, 128, H, W), dt.float32, kind="ExternalOutput").ap()

    with TileContext(nc) as tc:
        with (
            tc.tile_pool(name="const", bufs=1) as constp,
            tc.tile_pool(name="smpool", bufs=1) as smpool,
            tc.tile_pool(name="io", bufs=2) as iop,
            tc.tile_pool(name="tmp", bufs=1) as tmp,
            tc.tile_pool(name="outp", bufs=6) as outp,
            tc.tile_pool(name="psum", bufs=8, space="PSUM") as psp,
        ):
            id_sb = constp.tile([128, 128], DT)
            nc.sync.dma_start(out=id_sb, in_=ident)
            w_sb = constp.tile([128, 2, 9 * 2 * 128], DT)
            b_sb = constp.tile([128, 2], dt.float32)

            def load_consts():
                # emitted after the first slab's input DMAs: conv weights are
                # not needed until well into the first slab
                nc.sync.dma_start(out=w_sb[:, 0], in_=wts[0])
                nc.sync.dma_start(out=w_sb[:, 1], in_=wts[1])
                nc.sync.dma_start(out=b_sb, in_=bias)

            if loop is not None:
                # looped timing variant: weights must be resident before the
                # hardware loop body runs
                load_consts()

            n_sm = min(unroll, 2) if loop is not None else 1
            sms = []
            for i in range(n_sm):
                sm = smpool.tile([128, 2, HP, WP], DT, name=f"sm{i}")
                # zero only the 2-wide pad ring; the interior is fully
                # rewritten every pass
                nc.vector.memset(sm[:, :, 0:2, :], 0.0)
                nc.vector.memset(sm[:, :, HP - 2 : HP, :], 0.0)
                nc.vector.memset(sm[:, :, 2 : HP - 2, 0:2], 0.0)
                nc.vector.memset(sm[:, :, 2 : HP - 2, WP - 2 : WP], 0.0)
                sms.append(sm)

            def merge(sm, st):
                # deferred merge: combine slab st's DVE partial (acc) with the
                # Pool partial (m2+m8) one slab later, giving Pool a full slab
                # of slack before its outputs are consumed.
                r0, nr, acc, m2, m8 = st
                sm_out = sm[:, :, 2 + r0 : 2 + r0 + nr, 2 : W + 2]
                Bm = tmp.tile([128, 2, nr, W], DT, name="Bm", bufs=1)
                nc.vector.tensor_add(Bm, m2, m8)
                nc.vector.tensor_add(sm_out, acc, Bm)

            def smooth(sm, r0, nr, first=False, prev=None, flush=None,
                       s5_on_pe=None):
                s5_on_pe = pe_s5 if s5_on_pe is None else s5_on_pe
                xs = iop.tile([128, 2, nr + 4, WP], DT, name="xs")
                nc.sync.dma_start(out=xs, in_=xp[:, :, r0 : r0 + nr + 4, :])
                cp = iop.tile([128, 6, nr, W], DT, name="cp")
                nc.sync.dma_start(out=cp, in_=cpl[:, :, r0 : r0 + nr, :])
                if first:
                    load_consts()

                def cpm(m):
                    i = MS.index(m)
                    return cp[:, i : i + 1].to_broadcast([128, 2, nr, W])

                P0 = xs[:, :, :, 2 : W + 2]
                P1 = tmp.tile([128, 2, nr + 4, W], DT, name="P1", bufs=2)
                nc.vector.tensor_add(P1, xs[:, :, :, 1 : W + 1], xs[:, :, :, 3 : W + 3])
                P2 = tmp.tile([128, 2, nr + 4, W], DT, name="P2", bufs=2)
                nc.vector.tensor_add(P2, xs[:, :, :, 0:W], xs[:, :, :, 4 : W + 4])

                if prev is not None:
                    # merge the PREVIOUS slab now that Pool has had a slab of
                    # runway, then release its conv chunks
                    merge(sm, prev)
                    if flush is not None:
                        flush(prev[0] + prev[1])

                ctr = lambda P: P[:, :, 2 : nr + 2]
                u1 = lambda P: P[:, :, 1 : nr + 1]
                d1 = lambda P: P[:, :, 3 : nr + 3]
                u2 = lambda P: P[:, :, 0:nr]
                d2 = lambda P: P[:, :, 4 : nr + 4]

                # --- S2/S8 + their multiplies on Pool (gpsimd), S5 on PE ---
                # pool_ops: number of the ops [S2, S8, m2, m8] offloaded to
                # the Pool engine (rest stay on DVE).
                if pool_ops:
                    def eng(i):
                        return nc.gpsimd if i < pool_ops else nc.vector

                    S2 = tmp.tile([128, 2, nr, W], DT, name="S2", bufs=1)
                    eng(0).tensor_add(S2, u1(P1), d1(P1))
                    S8 = tmp.tile([128, 2, nr, W], DT, name="S8", bufs=1)
                    eng(1).tensor_add(S8, u2(P2), d2(P2))
                    m2 = tmp.tile([128, 2, nr, W], DT, name="m2", bufs=2)
                    eng(2).tensor_mul(m2, S2, cpm(2))
                    m8 = tmp.tile([128, 2, nr, W], DT, name="m8", bufs=2)
                    eng(3).tensor_mul(m8, S8, cpm(8))

                def pe_stencil(name, taps):
                    # k-tap stencil sum on TensorE via identity matmuls
                    # accumulated in PSUM, evacuated to SBUF by ScalarE.
                    out_t = tmp.tile([128, 2, nr, W], DT, name=name, bufs=2)
                    for ct in range(2):
                        for rk in range(nr // CHUNK):
                            rs = CHUNK * rk
                            pc5 = psp.tile(
                                [128, CHUNK, W], dt.float32, name="pc5", bufs=2
                            )
                            for j, Pv in enumerate(taps):
                                nc.tensor.matmul(
                                    pc5,
                                    id_sb,
                                    Pv[:, ct, rs : rs + CHUNK, :],
                                    start=(j == 0),
                                    stop=(j == len(taps) - 1),
                                )
                            nc.scalar.activation(
                                out_t[:, ct, rs : rs + CHUNK, :],
                                pc5,
                                mybir.ActivationFunctionType.Copy,
                            )
                    return out_t

                if s5_on_pe:
                    # S5 = (P1[h-2]+P1[h+2]) + (P2[h-1]+P2[h+1])
                    S5 = pe_stencil("S5", (u2(P1), d2(P1), u1(P2), d1(P2)))
                if pe_s2 and s5_on_pe:
                    S2 = pe_stencil("S2p", (u1(P1), d1(P1)))
                if pe_s8 and s5_on_pe:
                    S8 = pe_stencil("S8p", (u2(P2), d2(P2)))

                # --- S1/S4 builds + apply chain on DVE ---
                Q = tmp.tile([128, 2, nr, W], DT, name="Q", bufs=2)
                nc.vector.tensor_add(Q, u1(P0), d1(P0))
                S1 = tmp.tile([128, 2, nr, W], DT, name="S1", bufs=2)
                nc.vector.tensor_add(S1, Q, ctr(P1))
                Q2 = tmp.tile([128, 2, nr, W], DT, name="Q", bufs=2)
                nc.vector.tensor_add(Q2, u2(P0), d2(P0))
                S4 = tmp.tile([128, 2, nr, W], DT, name="S4", bufs=2)
                nc.vector.tensor_add(S4, Q2, ctr(P2))

                if not pool_ops:
                    if not (pe_s2 and s5_on_pe):
                        S2 = tmp.tile([128, 2, nr, W], DT, name="S2v", bufs=2)
                        nc.vector.tensor_add(S2, u1(P1), d1(P1))
                    if not (pe_s8 and s5_on_pe):
                        S8 = tmp.tile([128, 2, nr, W], DT, name="S8v", bufs=2)
                        nc.vector.tensor_add(S8, u2(P2), d2(P2))
                if not s5_on_pe:
                    S5 = tmp.tile([128, 2, nr, W], DT, name="S5v", bufs=2)
                    nc.vector.tensor_add(S5, u2(P1), d2(P1))
                    Qb = tmp.tile([128, 2, nr, W], DT, name="Qb", bufs=2)
                    nc.vector.tensor_add(Qb, u1(P2), d1(P2))
                    nc.vector.tensor_add(S5, S5, Qb)

                acc = tmp.tile([128, 2, nr, W], DT, name="acc", bufs=2)
                nc.vector.tensor_mul(acc, ctr(P0), cpm(0))
                t = tmp.tile([128, 2, nr, W], DT, name="t", bufs=3)
                nc.vector.tensor_mul(t, S1, cpm(1))
                nc.vector.tensor_add(acc, acc, t)
                t4 = tmp.tile([128, 2, nr, W], DT, name="t", bufs=3)
                nc.vector.tensor_mul(t4, S4, cpm(4))
                nc.vector.tensor_add(acc, acc, t4)
                t5 = tmp.tile([128, 2, nr, W], DT, name="t", bufs=3)
                nc.vector.tensor_mul(t5, S5, cpm(5))
                nc.vector.tensor_add(acc, acc, t5)

                if pool_ops:
                    return (r0, nr, acc, m2, m8)
                sm_out = sm[:, :, 2 + r0 : 2 + r0 + nr, 2 : W + 2]
                t2 = tmp.tile([128, 2, nr, W], DT, name="t", bufs=3)
                nc.vector.tensor_mul(t2, S2, cpm(2))
                nc.vector.tensor_add(acc, acc, t2)
                t8 = tmp.tile([128, 2, nr, W], DT, name="t", bufs=3)
                nc.vector.tensor_mul(t8, S8, cpm(8))
                nc.vector.tensor_add(sm_out, acc, t8)
                return None

            def conv_group(sm, rrs):
                # rrs: output-row starts whose sm dependencies are met; one
                # weight load serves len(rrs) matmuls.
                for oi in range(2):
                    pcs = [
                        psp.tile([128, CHUNK, W], dt.float32, name="pc", bufs=6)
                        for _ in rrs
                    ]
                    for idx in range(18):
                        ki, q = idx // 9, idx % 9
                        dh, dw = OFFS[q // 3], OFFS[q % 3]
                        lhsT = w_sb[:, ki, (q * 2 + oi) * 128 : (q * 2 + oi + 1) * 128]
                        for j, rr in enumerate(rrs):
                            rhs = sm[
                                :, ki, 2 + rr + dh : 2 + rr + CHUNK + dh, 2 + dw : 2 + dw + W
                            ]
                            nc.tensor.matmul(
                                pcs[j], lhsT, rhs, start=(idx == 0), stop=(idx == 17)
                            )
                    for j, rr in enumerate(rrs):
                        ob = outp.tile([128, CHUNK, W], dt.float32, name="ob")
                        nc.scalar.activation(
                            ob,
                            pcs[j],
                            mybir.ActivationFunctionType.Relu,
                            bias=b_sb[:, oi : oi + 1],
                            scale=1.0,
                        )
                        nc.sync.dma_start(out=y[oi, :, rr : rr + CHUNK, :], in_=ob)

            def body(sm, first=False):
                # conv rows rr..rr+3 read sm rows rr..rr+7 (interior rr-2..rr+5):
                # a chunk is ready once smoothing covers row rr+5. Batch
                # flushes (>=3 chunks) so one weight pass serves several
                # matmul columns.
                pending = list(range(0, H, CHUNK))
                emitted = [0]

                def flush(upto):
                    ready = [rr for rr in pending if rr + 6 <= upto or upto >= H]
                    if not ready:
                        return
                    thresh = flush_min_first if emitted[0] == 0 else flush_min
                    if upto < H and len(ready) < thresh:
                        return
                    for rr in ready:
                        pending.remove(rr)
                    emitted[0] += len(ready)
                    conv_group(sm, ready)

                prev = None
                for si, (r0, nr) in enumerate(SLABS):
                    prev = smooth(
                        sm, r0, nr, first=first and si == 0, prev=prev, flush=flush,
                        s5_on_pe=pe_s5 and si < len(SLABS) - s5_dve_tail,
                    )
                    if prev is None:
                        flush(r0 + nr)
                if prev is not None:
                    merge(sm, prev)
                flush(H)
                assert not pending

            if loop is not None:
                assert loop % n_sm == 0
                with tc.For_i(0, loop // n_sm, 1):
                    for i in range(n_sm):
                        body(sms[i], first=False)
            else:
                for r in range(repeats):
                    body(sms[0], first=(r == 0))

    nc.compile()
    return nc


def _prep(inputs, fp16=False):
    NPDT = np.float16 if fp16 else ml_dtypes.bfloat16
    x = np.asarray(inputs["x"], np.float32)
    pm = np.asarray(inputs["perspective_map"], np.float32)
    co = np.asarray(inputs["sigma_coeffs"], np.float32)
    Wc = np.asarray(inputs["conv_w"], np.float32)
    bb = np.asarray(inputs["conv_b"], np.float32)

    # per-pixel coefficient planes (host): c_m = t^m / Z, replicated over partitions
    p = pm[:, 0]  # [B,H,W]
    sigma = co[0] * p**3 + co[1] * p**2 + co[2] * p + co[3]
    sigma = np.maximum(sigma, 0.5)
    t = np.exp(-1.0 / (2.0 * sigma * sigma))
    Z = 1 + 4 * t + 4 * t**2 + 4 * t**4 + 8 * t**5 + 4 * t**8
    cm = np.stack([(t**m) / Z for m in MS], axis=1).astype(NPDT)  # [B,6,H,W]
    cpl = np.ascontiguousarray(np.broadcast_to(cm[:, None], (B, 128, 6, H, W)))

    # zero-padded input: [B, 128(part), 2(ct), HP, WP]
    xpad = np.zeros((B, 128, 2, HP, WP), NPDT)
    xpad[:, :, :, 2 : H + 2, 2 : W + 2] = (
        x.astype(NPDT).reshape(B, 2, 128, H, W).transpose(0, 2, 1, 3, 4)
    )

    # conv weights: lhsT layout [ki, 128(i), q, oi, 128(o)]
    Wt = Wc.transpose(1, 0, 2, 3).astype(NPDT)  # [I, O, kh, kw]
    wts = np.empty((2, 128, 9, 2, 128), NPDT)
    for ki in range(2):
        for q in range(9):
            kh, kw = q // 3, q % 3
            for oi in range(2):
                wts[ki, :, q, oi, :] = Wt[
                    ki * 128 : (ki + 1) * 128, oi * 128 : (oi + 1) * 128, kh, kw
                ]
    wts = wts.reshape(2, 128, 9 * 2 * 128)
    bias_h = np.ascontiguousarray(bb.reshape(2, 128).T.astype(np.float32))  # [128, 2]
    ident = np.eye(128, dtype=NPDT)

    return [
        {"xp": xpad[b], "cpl": cpl[b], "wts": wts, "bias": bias_h, "ident": ident}
        for b in range(B)
    ]


def _get_nc(repeats=1, loop=None, **kw):
    key = ("nc", repeats, loop, tuple(sorted(kw.items())))
    if key not in _cache:
        _cache[key] = _build(repeats, loop, **kw)
    return _cache[key]


def run(inputs, trace=False, **kw):
    from concourse.bass_utils import run_bass_kernel_spmd

    nc = _get_nc()
    in_maps = _prep(inputs)
    res = run_bass_kernel_spmd(nc, in_maps, core_ids=list(range(B)), trace=trace, **kw)
    out = np.stack([r["y"].reshape(C, H, W) for r in res.results]).astype(np.float32)
    return out, res


def kernel(**inputs):
    out, _ = run(inputs)
    return out


# revision 36
# speedup vs baseline: 1.2664x; 1.2664x over previous
"""Trainium2 Bass kernel for BasicPGCBlock:
   per-pixel Gaussian smoothing (5x5, sigma = cubic(perspective)) -> dilated 3x3 conv (256->256) + bias + ReLU.

Sharding: data-parallel over batch, 1 image per NeuronCore (8 cores).

Math: the per-pixel 5x5 kernel w(u,v) = exp(-(u^2+v^2)/(2 s^2)) / Z factors through
t = exp(-1/(2 s^2)):  w(u,v) = t^(u^2+v^2) / Z, and u^2+v^2 in {0,1,2,4,5,8}.
So smoothed = sum_m c_m * S_m with c_m = t^m / Z (host-computed per-pixel planes,
replicated across partitions) and S_m = fixed 0/1 stencil sums of x built from
shifted adds.

Engine balance (HW-measured):
  PE:   dilated conv (18 accumulating matmuls per 4-row chunk per half-out) +
        S5 (4 taps) and S2 (2 taps) built via identity matmuls in PSUM.
  DVE:  P1/P2 horizontal sums, S1/S4/S8 vertical builds, the 6-term apply
        chain (bf16 2x mode, ~0.56 ns/elem).
  Act:  S5/S2 PSUM evacuation, conv bias+ReLU evacuation.
  Pool (gpsimd): UNUSED — measured ~10x slower than the cost model claims
        for strided/broadcast tensor ops on real TRN2; offloading any
        elementwise to it regressed wall time by ~70-150us.

All smoothing in bf16 (DVE 2x mode; fp16 measured ~4% slower end-to-end),
conv in bf16 with f32 PSUM accumulation. 8-row slabs pipeline smoothing
against the conv; conv chunks flush as soon as their sm rows settle.
"""

import sys

sys.path.insert(0, "/opt/trn_rl_repo")

import numpy as np
import ml_dtypes

B, C, H, W = 8, 256, 96, 96
HP, WP = H + 4, W + 4          # zero-padded by 2 on each side
CHUNK = 4                      # conv output rows per matmul (N = 4*96 = 384 <= 512)
OFFS = (-2, 0, 2)              # dilated conv offsets
MS = (0, 1, 2, 4, 5, 8)        # exponents of t present in the 5x5 kernel

_cache = {}


def _slabs(n=8):
    assert H % n == 0
    return tuple((r, n) for r in range(0, H, n))


def _build(repeats=1, loop=None, fp16=False, pool_ops=0, pe_s5=True,
           pe_s2=True, pe_s8=False, slabs=None, unroll=2, flush_min=2,
           flush_min_first=2, s5_dve_tail=0, pingpong=False):
    import concourse.mybir as mybir
    from concourse import bacc
    from concourse.tile import TileContext

    dt = mybir.dt
    DT = dt.float16 if fp16 else dt.bfloat16
    SLABS = slabs if slabs is not None else _slabs()
    nc = bacc.Bacc("TRN2", target_bir_lowering=False, debug=False)

    xp = nc.dram_tensor("xp", (128, 2, HP, WP), DT, kind="ExternalInput").ap()
    cpl = nc.dram_tensor("cpl", (128, 6, H, W), DT, kind="ExternalInput").ap()
    wts = nc.dram_tensor("wts", (2, 128, 9 * 2 * 128), DT, kind="ExternalInput").ap()
    bias = nc.dram_tensor("bias", (128, 2), dt.float32, kind="ExternalInput").ap()
    ident = nc.dram_tensor("ident", (128, 128), DT, kind="ExternalInput").ap()
    y = nc.dram_tensor("y", (2, 128, H, W), dt.float32, kind="ExternalOutput").ap()

    with TileContext(nc) as tc:
        with (
            tc.tile_pool(name="const", bufs=1) as constp,
            tc.tile_pool(name="smpool", bufs=1) as smpool,
            tc.tile_pool(name="io", bufs=2) as iop,
            tc.tile_pool(name="tmp", bufs=1) as tmp,
            tc.tile_pool(name="outp", bufs=6) as outp,
            tc.tile_pool(name="psum", bufs=8, space="PSUM") as psp,
        ):
            id_sb = constp.tile([128, 128], DT)
            nc.sync.dma_start(out=id_sb, in_=ident)
            w_sb = constp.tile([128, 2, 9 * 2 * 128], DT)
            b_sb = constp.tile([128, 2], dt.float32)

            def load_consts():
                # emitted after the first slab's input DMAs: conv weights are
                # not needed until well into the first slab
                nc.sync.dma_start(out=w_sb[:, 0], in_=wts[0])
                nc.sync.dma_start(out=w_sb[:, 1], in_=wts[1])
                nc.sync.dma_start(out=b_sb, in_=bias)

            if loop is not None:
                # looped timing variant: weights must be resident before the
                # hardware loop body runs
                load_consts()

            n_sm = min(unroll, 2) if (loop is not None and pingpong) else 1
            sms = []
            for i in range(n_sm):
                sm = smpool.tile([128, 2, HP, WP], DT, name=f"sm{i}")
                # zero only the 2-wide pad ring; the interior is fully
                # rewritten every pass
                nc.vector.memset(sm[:, :, 0:2, :], 0.0)
                nc.vector.memset(sm[:, :, HP - 2 : HP, :], 0.0)
                nc.vector.memset(sm[:, :, 2 : HP - 2, 0:2], 0.0)
                nc.vector.memset(sm[:, :, 2 : HP - 2, WP - 2 : WP], 0.0)
                sms.append(sm)

            def merge(sm, st):
                # deferred merge: combine slab st's DVE partial (acc) with the
                # Pool partial (m2+m8) one slab later, giving Pool a full slab
                # of slack before its outputs are consumed.
                r0, nr, acc, m2, m8 = st
                sm_out = sm[:, :, 2 + r0 : 2 + r0 + nr, 2 : W + 2]
                Bm = tmp.tile([128, 2, nr, W], DT, name="Bm", bufs=1)
                nc.vector.tensor_add(Bm, m2, m8)
                nc.vector.tensor_add(sm_out, acc, Bm)

            def smooth(sm, r0, nr, first=False, prev=None, flush=None,
                       s5_on_pe=None):
                s5_on_pe = pe_s5 if s5_on_pe is None else s5_on_pe
                xs = iop.tile([128, 2, nr + 4, WP], DT, name="xs")
                nc.sync.dma_start(out=xs, in_=xp[:, :, r0 : r0 + nr + 4, :])
                cp = iop.tile([128, 6, nr, W], DT, name="cp")
                nc.sync.dma_start(out=cp, in_=cpl[:, :, r0 : r0 + nr, :])
                if first:
                    load_consts()

                def cpm(m):
                    i = MS.index(m)
                    return cp[:, i : i + 1].to_broadcast([128, 2, nr, W])

                P0 = xs[:, :, :, 2 : W + 2]
                P1 = tmp.tile([128, 2, nr + 4, W], DT, name="P1", bufs=2)
                nc.vector.tensor_add(P1, xs[:, :, :, 1 : W + 1], xs[:, :, :, 3 : W + 3])
                P2 = tmp.tile([128, 2, nr + 4, W], DT, name="P2", bufs=2)
                nc.vector.tensor_add(P2, xs[:, :, :, 0:W], xs[:, :, :, 4 : W + 4])

                if prev is not None:
                    # merge the PREVIOUS slab now that Pool has had a slab of
                    # runway, then release its conv chunks
                    merge(sm, prev)
                    if flush is not None:
                        flush(prev[0] + prev[1])

                ctr = lambda P: P[:, :, 2 : nr + 2]
                u1 = lambda P: P[:, :, 1 : nr + 1]
                d1 = lambda P: P[:, :, 3 : nr + 3]
                u2 = lambda P: P[:, :, 0:nr]
                d2 = lambda P: P[:, :, 4 : nr + 4]

                # --- S2/S8 + their multiplies on Pool (gpsimd), S5 on PE ---
                # pool_ops: number of the ops [S2, S8, m2, m8] offloaded to
                # the Pool engine (rest stay on DVE).
                if pool_ops:
                    def eng(i):
                        return nc.gpsimd if i < pool_ops else nc.vector

                    S2 = tmp.tile([128, 2, nr, W], DT, name="S2", bufs=1)
                    eng(0).tensor_add(S2, u1(P1), d1(P1))
                    S8 = tmp.tile([128, 2, nr, W], DT, name="S8", bufs=1)
                    eng(1).tensor_add(S8, u2(P2), d2(P2))
                    m2 = tmp.tile([128, 2, nr, W], DT, name="m2", bufs=2)
                    eng(2).tensor_mul(m2, S2, cpm(2))
                    m8 = tmp.tile([128, 2, nr, W], DT, name="m8", bufs=2)
                    eng(3).tensor_mul(m8, S8, cpm(8))

                def pe_stencil(name, taps):
                    # k-tap stencil sum on TensorE via identity matmuls
                    # accumulated in PSUM, evacuated to SBUF by ScalarE.
                    out_t = tmp.tile([128, 2, nr, W], DT, name=name, bufs=2)
                    for ct in range(2):
                        for rk in range(nr // CHUNK):
                            rs = CHUNK * rk
                            pc5 = psp.tile(
                                [128, CHUNK, W], dt.float32, name="pc5", bufs=2
                            )
                            for j, Pv in enumerate(taps):
                                nc.tensor.matmul(
                                    pc5,
                                    id_sb,
                                    Pv[:, ct, rs : rs + CHUNK, :],
                                    start=(j == 0),
                                    stop=(j == len(taps) - 1),
                                )
                            nc.scalar.activation(
                                out_t[:, ct, rs : rs + CHUNK, :],
                                pc5,
                                mybir.ActivationFunctionType.Copy,
                            )
                    return out_t

                if s5_on_pe:
                    # S5 = (P1[h-2]+P1[h+2]) + (P2[h-1]+P2[h+1])
                    S5 = pe_stencil("S5", (u2(P1), d2(P1), u1(P2), d1(P2)))
                if pe_s2 and s5_on_pe:
                    S2 = pe_stencil("S2p", (u1(P1), d1(P1)))
                if pe_s8 and s5_on_pe:
                    S8 = pe_stencil("S8p", (u2(P2), d2(P2)))

                # --- S1/S4 builds + apply chain on DVE ---
                Q = tmp.tile([128, 2, nr, W], DT, name="Q", bufs=2)
                nc.vector.tensor_add(Q, u1(P0), d1(P0))
                S1 = tmp.tile([128, 2, nr, W], DT, name="S1", bufs=2)
                nc.vector.tensor_add(S1, Q, ctr(P1))
                Q2 = tmp.tile([128, 2, nr, W], DT, name="Q", bufs=2)
                nc.vector.tensor_add(Q2, u2(P0), d2(P0))
                S4 = tmp.tile([128, 2, nr, W], DT, name="S4", bufs=2)
                nc.vector.tensor_add(S4, Q2, ctr(P2))

                if not pool_ops:
                    if not (pe_s2 and s5_on_pe):
                        S2 = tmp.tile([128, 2, nr, W], DT, name="S2v", bufs=2)
                        nc.vector.tensor_add(S2, u1(P1), d1(P1))
                    if not (pe_s8 and s5_on_pe):
                        S8 = tmp.tile([128, 2, nr, W], DT, name="S8v", bufs=2)
                        nc.vector.tensor_add(S8, u2(P2), d2(P2))
                if not s5_on_pe:
                    S5 = tmp.tile([128, 2, nr, W], DT, name="S5v", bufs=2)
                    nc.vector.tensor_add(S5, u2(P1), d2(P1))
                    Qb = tmp.tile([128, 2, nr, W], DT, name="Qb", bufs=2)
                    nc.vector.tensor_add(Qb, u1(P2), d1(P2))
                    nc.vector.tensor_add(S5, S5, Qb)

                acc = tmp.tile([128, 2, nr, W], DT, name="acc", bufs=2)
                nc.vector.tensor_mul(acc, ctr(P0), cpm(0))
                t = tmp.tile([128, 2, nr, W], DT, name="t", bufs=3)
                nc.vector.tensor_mul(t, S1, cpm(1))
                nc.vector.tensor_add(acc, acc, t)
                t4 = tmp.tile([128, 2, nr, W], DT, name="t", bufs=3)
                nc.vector.tensor_mul(t4, S4, cpm(4))
                nc.vector.tensor_add(acc, acc, t4)
                t5 = tmp.tile([128, 2, nr, W], DT, name="t", bufs=3)
                nc.vector.tensor_mul(t5, S5, cpm(5))
                nc.vector.tensor_add(acc, acc, t5)

                if pool_ops:
                    return (r0, nr, acc, m2, m8)
                sm_out = sm[:, :, 2 + r0 : 2 + r0 + nr, 2 : W + 2]
                t2 = tmp.tile([128, 2, nr, W], DT, name="t", bufs=3)
                nc.vector.tensor_mul(t2, S2, cpm(2))
                nc.vector.tensor_add(acc, acc, t2)
                t8 = tmp.tile([128, 2, nr, W], DT, name="t", bufs=3)
                nc.vector.tensor_mul(t8, S8, cpm(8))
                nc.vector.tensor_add(sm_out, acc, t8)
                return None

            def conv_group(sm, rrs):
                # rrs: output-row starts whose sm dependencies are met; one
                # weight load serves len(rrs) matmuls.
                for oi in range(2):
                    pcs = [
                        psp.tile([128, CHUNK, W], dt.float32, name="pc", bufs=6)
                        for _ in rrs
                    ]
                    for idx in range(18):
                        ki, q = idx // 9, idx % 9
                        dh, dw = OFFS[q // 3], OFFS[q % 3]
                        lhsT = w_sb[:, ki, (q * 2 + oi) * 128 : (q * 2 + oi + 1) * 128]
                        for j, rr in enumerate(rrs):
                            rhs = sm[
                                :, ki, 2 + rr + dh : 2 + rr + CHUNK + dh, 2 + dw : 2 + dw + W
                            ]
                            nc.tensor.matmul(
                                pcs[j], lhsT, rhs, start=(idx == 0), stop=(idx == 17)
                            )
                    for j, rr in enumerate(rrs):
                        ob = outp.tile([128, CHUNK, W], dt.float32, name="ob")
                        nc.scalar.activation(
                            ob,
                            pcs[j],
                            mybir.ActivationFunctionType.Relu,
                            bias=b_sb[:, oi : oi + 1],
                            scale=1.0,
                        )
                        nc.sync.dma_start(out=y[oi, :, rr : rr + CHUNK, :], in_=ob)

            def body(sm, first=False):
                # conv rows rr..rr+3 read sm rows rr..rr+7 (interior rr-2..rr+5):
                # a chunk is ready once smoothing covers row rr+5. Batch
                # flushes (>=3 chunks) so one weight pass serves several
                # matmul columns.
                pending = list(range(0, H, CHUNK))
                emitted = [0]

                def flush(upto):
                    ready = [rr for rr in pending if rr + 6 <= upto or upto >= H]
                    if not ready:
                        return
                    thresh = flush_min_first if emitted[0] == 0 else flush_min
                    if upto < H and len(ready) < thresh:
                        return
                    for rr in ready:
                        pending.remove(rr)
                    emitted[0] += len(ready)
                    conv_group(sm, ready)

                prev = None
                for si, (r0, nr) in enumerate(SLABS):
                    prev = smooth(
                        sm, r0, nr, first=first and si == 0, prev=prev, flush=flush,
                        s5_on_pe=pe_s5 and si < len(SLABS) - s5_dve_tail,
                    )
                    if prev is None:
                        flush(r0 + nr)
                if prev is not None:
                    merge(sm, prev)
                flush(H)
                assert not pending

            if loop is not None:
                assert loop % unroll == 0
                with tc.For_i(0, loop // unroll, 1):
                    for i in range(unroll):
                        body(sms[i % n_sm], first=False)
            else:
                for r in range(repeats):
                    body(sms[0], first=(r == 0))

    nc.compile()
    return nc


def _prep(inputs, fp16=False):
    NPDT = np.float16 if fp16 else ml_dtypes.bfloat16
    x = np.asarray(inputs["x"], np.float32)
    pm = np.asarray(inputs["perspective_map"], np.float32)
    co = np.asarray(inputs["sigma_coeffs"], np.float32)
    Wc = np.asarray(inputs["conv_w"], np.float32)
    bb = np.asarray(inputs["conv_b"], np.float32)

    # per-pixel coefficient planes (host): c_m = t^m / Z, replicated over partitions
    p = pm[:, 0]  # [B,H,W]
    sigma = co[0] * p**3 + co[1] * p**2 + co[2] * p + co[3]
    sigma = np.maximum(sigma, 0.5)
    t = np.exp(-1.0 / (2.0 * sigma * sigma))
    Z = 1 + 4 * t + 4 * t**2 + 4 * t**4 + 8 * t**5 + 4 * t**8
    cm = np.stack([(t**m) / Z for m in MS], axis=1).astype(NPDT)  # [B,6,H,W]
    cpl = np.ascontiguousarray(np.broadcast_to(cm[:, None], (B, 128, 6, H, W)))

    # zero-padded input: [B, 128(part), 2(ct), HP, WP]
    xpad = np.zeros((B, 128, 2, HP, WP), NPDT)
    xpad[:, :, :, 2 : H + 2, 2 : W + 2] = (
        x.astype(NPDT).reshape(B, 2, 128, H, W).transpose(0, 2, 1, 3, 4)
    )

    # conv weights: lhsT layout [ki, 128(i), q, oi, 128(o)]
    Wt = Wc.transpose(1, 0, 2, 3).astype(NPDT)  # [I, O, kh, kw]
    wts = np.empty((2, 128, 9, 2, 128), NPDT)
    for ki in range(2):
        for q in range(9):
            kh, kw = q // 3, q % 3
            for oi in range(2):
                wts[ki, :, q, oi, :] = Wt[
                    ki * 128 : (ki + 1) * 128, oi * 128 : (oi + 1) * 128, kh, kw
                ]
    wts = wts.reshape(2, 128, 9 * 2 * 128)
    bias_h = np.ascontiguousarray(bb.reshape(2, 128).T.astype(np.float32))  # [128, 2]
    ident = np.eye(128, dtype=NPDT)

    return [
        {"xp": xpad[b], "cpl": cpl[b], "wts": wts, "bias": bias_h, "ident": ident}
        for b in range(B)
    ]


def _get_nc(repeats=1, loop=None, **kw):
    key = ("nc", repeats, loop, tuple(sorted(kw.items())))
    if key not in _cache:
        _cache[key] = _build(repeats, loop, **kw)
    return _cache[key]


def run(inputs, trace=False, **kw):
    from concourse.bass_utils import run_bass_kernel_spmd

    nc = _get_nc()
    in_maps = _prep(inputs)
    res = run_bass_kernel_spmd(nc, in_maps, core_ids=list(range(B)), trace=trace, **kw)
    out = np.stack([r["y"].reshape(C, H, W) for r in res.results]).astype(np.float32)
    return out, res


def kernel(**inputs):
    out, _ = run(inputs)
    return out


# revision 37
# speedup vs baseline: 1.9587x; 1.5467x over previous
"""Trainium2 Bass kernel for BasicPGCBlock:
   per-pixel Gaussian smoothing (5x5, sigma = cubic(perspective)) -> dilated 3x3 conv (256->256) + bias + ReLU.

Sharding: data-parallel over batch, 1 image per NeuronCore (8 cores).

Math: the per-pixel 5x5 kernel w(u,v) = exp(-(u^2+v^2)/(2 s^2)) / Z factors through
t = exp(-1/(2 s^2)):  w(u,v) = t^(u^2+v^2) / Z, and u^2+v^2 in {0,1,2,4,5,8}.
So smoothed = sum_m c_m * S_m with c_m = t^m / Z (host-computed per-pixel planes,
replicated across partitions) and S_m = fixed 0/1 stencil sums of x built from
shifted adds.

Engine balance (HW-measured):
  PE:   dilated conv (18 accumulating matmuls per 4-row chunk per half-out) +
        S5 (4 taps) and S2 (2 taps) built via identity matmuls in PSUM.
  DVE:  P1/P2 horizontal sums, S1/S4/S8 vertical builds, the 6-term apply
        chain (bf16 2x mode, ~0.56 ns/elem).
  Act:  S5/S2 PSUM evacuation, conv bias+ReLU evacuation.
  Pool (gpsimd): UNUSED — measured ~10x slower than the cost model claims
        for strided/broadcast tensor ops on real TRN2; offloading any
        elementwise to it regressed wall time by ~70-150us.

All smoothing in bf16 (DVE 2x mode; fp16 measured ~4% slower end-to-end),
conv in bf16 with f32 PSUM accumulation. 8-row slabs pipeline smoothing
against the conv; conv chunks flush as soon as their sm rows settle.
"""

import sys

sys.path.insert(0, "/opt/trn_rl_repo")

import numpy as np
import ml_dtypes

B, C, H, W = 8, 256, 96, 96
HP, WP = H + 4, W + 4          # zero-padded by 2 on each side
CHUNK = 4                      # conv output rows per matmul (N = 4*96 = 384 <= 512)
OFFS = (-2, 0, 2)              # dilated conv offsets
MS = (0, 1, 2, 4, 5, 8)        # exponents of t present in the 5x5 kernel

_cache = {}


def _slabs(n=8):
    assert H % n == 0
    return tuple((r, n) for r in range(0, H, n))


def _build(repeats=1, loop=None, fp16=False, pool_ops=0, pe_s5=True,
           pe_s2=True, pe_s8=False, slabs=None, unroll=2, flush_min=2,
           flush_min_first=2, s5_dve_tail=0, pingpong=False, y_bf16=False):
    import concourse.mybir as mybir
    from concourse import bacc
    from concourse.tile import TileContext

    dt = mybir.dt
    DT = dt.float16 if fp16 else dt.bfloat16
    SLABS = slabs if slabs is not None else _slabs()
    nc = bacc.Bacc("TRN2", target_bir_lowering=False, debug=False)

    xp = nc.dram_tensor("xp", (128, 2, HP, WP), DT, kind="ExternalInput").ap()
    cpl = nc.dram_tensor("cpl", (128, 6, H, W), DT, kind="ExternalInput").ap()
    wts = nc.dram_tensor("wts", (2, 128, 9 * 2 * 128), DT, kind="ExternalInput").ap()
    bias = nc.dram_tensor("bias", (128, 2), dt.float32, kind="ExternalInput").ap()
    ident = nc.dram_tensor("ident", (128, 128), DT, kind="ExternalInput").ap()
    YDT = dt.bfloat16 if y_bf16 else dt.float32
    y = nc.dram_tensor("y", (2, 128, H, W), YDT, kind="ExternalOutput").ap()

    with TileContext(nc) as tc:
        with (
            tc.tile_pool(name="const", bufs=1) as constp,
            tc.tile_pool(name="smpool", bufs=1) as smpool,
            tc.tile_pool(name="io", bufs=2) as iop,
            tc.tile_pool(name="tmp", bufs=1) as tmp,
            tc.tile_pool(name="outp", bufs=6) as outp,
            tc.tile_pool(name="psum", bufs=8, space="PSUM") as psp,
        ):
            id_sb = constp.tile([128, 128], DT)
            nc.sync.dma_start(out=id_sb, in_=ident)
            w_sb = constp.tile([128, 2, 9 * 2 * 128], DT)
            b_sb = constp.tile([128, 2], dt.float32)

            def load_consts():
                # emitted after the first slab's input DMAs: conv weights are
                # not needed until well into the first slab
                nc.sync.dma_start(out=w_sb[:, 0], in_=wts[0])
                nc.sync.dma_start(out=w_sb[:, 1], in_=wts[1])
                nc.sync.dma_start(out=b_sb, in_=bias)

            if loop is not None:
                # looped timing variant: weights must be resident before the
                # hardware loop body runs
                load_consts()

            n_sm = min(unroll, 2) if (loop is not None and pingpong) else 1
            sms = []
            for i in range(n_sm):
                sm = smpool.tile([128, 2, HP, WP], DT, name=f"sm{i}")
                # zero only the 2-wide pad ring; the interior is fully
                # rewritten every pass
                nc.vector.memset(sm[:, :, 0:2, :], 0.0)
                nc.vector.memset(sm[:, :, HP - 2 : HP, :], 0.0)
                nc.vector.memset(sm[:, :, 2 : HP - 2, 0:2], 0.0)
                nc.vector.memset(sm[:, :, 2 : HP - 2, WP - 2 : WP], 0.0)
                sms.append(sm)

            def merge(sm, st):
                # deferred merge: combine slab st's DVE partial (acc) with the
                # Pool partial (m2+m8) one slab later, giving Pool a full slab
                # of slack before its outputs are consumed.
                r0, nr, acc, m2, m8 = st
                sm_out = sm[:, :, 2 + r0 : 2 + r0 + nr, 2 : W + 2]
                Bm = tmp.tile([128, 2, nr, W], DT, name="Bm", bufs=1)
                nc.vector.tensor_add(Bm, m2, m8)
                nc.vector.tensor_add(sm_out, acc, Bm)

            def smooth(sm, r0, nr, first=False, prev=None, flush=None,
                       s5_on_pe=None):
                s5_on_pe = pe_s5 if s5_on_pe is None else s5_on_pe
                xs = iop.tile([128, 2, nr + 4, WP], DT, name="xs")
                nc.sync.dma_start(out=xs, in_=xp[:, :, r0 : r0 + nr + 4, :])
                cp = iop.tile([128, 6, nr, W], DT, name="cp")
                nc.sync.dma_start(out=cp, in_=cpl[:, :, r0 : r0 + nr, :])
                if first:
                    load_consts()

                def cpm(m):
                    i = MS.index(m)
                    return cp[:, i : i + 1].to_broadcast([128, 2, nr, W])

                P0 = xs[:, :, :, 2 : W + 2]
                P1 = tmp.tile([128, 2, nr + 4, W], DT, name="P1", bufs=2)
                nc.vector.tensor_add(P1, xs[:, :, :, 1 : W + 1], xs[:, :, :, 3 : W + 3])
                P2 = tmp.tile([128, 2, nr + 4, W], DT, name="P2", bufs=2)
                nc.vector.tensor_add(P2, xs[:, :, :, 0:W], xs[:, :, :, 4 : W + 4])

                if prev is not None:
                    # merge the PREVIOUS slab now that Pool has had a slab of
                    # runway, then release its conv chunks
                    merge(sm, prev)
                    if flush is not None:
                        flush(prev[0] + prev[1])

                ctr = lambda P: P[:, :, 2 : nr + 2]
                u1 = lambda P: P[:, :, 1 : nr + 1]
                d1 = lambda P: P[:, :, 3 : nr + 3]
                u2 = lambda P: P[:, :, 0:nr]
                d2 = lambda P: P[:, :, 4 : nr + 4]

                # --- S2/S8 + their multiplies on Pool (gpsimd), S5 on PE ---
                # pool_ops: number of the ops [S2, S8, m2, m8] offloaded to
                # the Pool engine (rest stay on DVE).
                if pool_ops:
                    def eng(i):
                        return nc.gpsimd if i < pool_ops else nc.vector

                    S2 = tmp.tile([128, 2, nr, W], DT, name="S2", bufs=1)
                    eng(0).tensor_add(S2, u1(P1), d1(P1))
                    S8 = tmp.tile([128, 2, nr, W], DT, name="S8", bufs=1)
                    eng(1).tensor_add(S8, u2(P2), d2(P2))
                    m2 = tmp.tile([128, 2, nr, W], DT, name="m2", bufs=2)
                    eng(2).tensor_mul(m2, S2, cpm(2))
                    m8 = tmp.tile([128, 2, nr, W], DT, name="m8", bufs=2)
                    eng(3).tensor_mul(m8, S8, cpm(8))

                def pe_stencil(name, taps):
                    # k-tap stencil sum on TensorE via identity matmuls
                    # accumulated in PSUM, evacuated to SBUF by ScalarE.
                    out_t = tmp.tile([128, 2, nr, W], DT, name=name, bufs=2)
                    for ct in range(2):
                        for rk in range(nr // CHUNK):
                            rs = CHUNK * rk
                            pc5 = psp.tile(
                                [128, CHUNK, W], dt.float32, name="pc5",
                                bufs=3 if pe_s8 else 2,
                            )
                            for j, Pv in enumerate(taps):
                                nc.tensor.matmul(
                                    pc5,
                                    id_sb,
                                    Pv[:, ct, rs : rs + CHUNK, :],
                                    start=(j == 0),
                                    stop=(j == len(taps) - 1),
                                )
                            nc.scalar.activation(
                                out_t[:, ct, rs : rs + CHUNK, :],
                                pc5,
                                mybir.ActivationFunctionType.Copy,
                            )
                    return out_t

                if s5_on_pe:
                    # S5 = (P1[h-2]+P1[h+2]) + (P2[h-1]+P2[h+1])
                    S5 = pe_stencil("S5", (u2(P1), d2(P1), u1(P2), d1(P2)))
                if pe_s2 and s5_on_pe:
                    S2 = pe_stencil("S2p", (u1(P1), d1(P1)))
                if pe_s8 and s5_on_pe:
                    S8 = pe_stencil("S8p", (u2(P2), d2(P2)))

                # --- S1/S4 builds + apply chain on DVE ---
                Q = tmp.tile([128, 2, nr, W], DT, name="Q", bufs=2)
                nc.vector.tensor_add(Q, u1(P0), d1(P0))
                S1 = tmp.tile([128, 2, nr, W], DT, name="S1", bufs=2)
                nc.vector.tensor_add(S1, Q, ctr(P1))
                Q2 = tmp.tile([128, 2, nr, W], DT, name="Q", bufs=2)
                nc.vector.tensor_add(Q2, u2(P0), d2(P0))
                S4 = tmp.tile([128, 2, nr, W], DT, name="S4", bufs=2)
                nc.vector.tensor_add(S4, Q2, ctr(P2))

                if not pool_ops:
                    if not (pe_s2 and s5_on_pe):
                        S2 = tmp.tile([128, 2, nr, W], DT, name="S2v", bufs=2)
                        nc.vector.tensor_add(S2, u1(P1), d1(P1))
                    if not (pe_s8 and s5_on_pe):
                        S8 = tmp.tile([128, 2, nr, W], DT, name="S8v", bufs=2)
                        nc.vector.tensor_add(S8, u2(P2), d2(P2))
                if not s5_on_pe:
                    S5 = tmp.tile([128, 2, nr, W], DT, name="S5v", bufs=2)
                    nc.vector.tensor_add(S5, u2(P1), d2(P1))
                    Qb = tmp.tile([128, 2, nr, W], DT, name="Qb", bufs=2)
                    nc.vector.tensor_add(Qb, u1(P2), d1(P2))
                    nc.vector.tensor_add(S5, S5, Qb)

                acc = tmp.tile([128, 2, nr, W], DT, name="acc", bufs=2)
                nc.vector.tensor_mul(acc, ctr(P0), cpm(0))
                t = tmp.tile([128, 2, nr, W], DT, name="t", bufs=3)
                nc.vector.tensor_mul(t, S1, cpm(1))
                nc.vector.tensor_add(acc, acc, t)
                t4 = tmp.tile([128, 2, nr, W], DT, name="t", bufs=3)
                nc.vector.tensor_mul(t4, S4, cpm(4))
                nc.vector.tensor_add(acc, acc, t4)
                t5 = tmp.tile([128, 2, nr, W], DT, name="t", bufs=3)
                nc.vector.tensor_mul(t5, S5, cpm(5))
                nc.vector.tensor_add(acc, acc, t5)

                if pool_ops:
                    return (r0, nr, acc, m2, m8)
                sm_out = sm[:, :, 2 + r0 : 2 + r0 + nr, 2 : W + 2]
                t2 = tmp.tile([128, 2, nr, W], DT, name="t", bufs=3)
                nc.vector.tensor_mul(t2, S2, cpm(2))
                nc.vector.tensor_add(acc, acc, t2)
                t8 = tmp.tile([128, 2, nr, W], DT, name="t", bufs=3)
                nc.vector.tensor_mul(t8, S8, cpm(8))
                nc.vector.tensor_add(sm_out, acc, t8)
                return None

            def conv_group(sm, rrs):
                # rrs: output-row starts whose sm dependencies are met; one
                # weight load serves len(rrs) matmuls.
                for oi in range(2):
                    pcs = [
                        psp.tile([128, CHUNK, W], dt.float32, name="pc",
                                 bufs=5 if pe_s8 else 6)
                        for _ in rrs
                    ]
                    for idx in range(18):
                        ki, q = idx // 9, idx % 9
                        dh, dw = OFFS[q // 3], OFFS[q % 3]
                        lhsT = w_sb[:, ki, (q * 2 + oi) * 128 : (q * 2 + oi + 1) * 128]
                        for j, rr in enumerate(rrs):
                            rhs = sm[
                                :, ki, 2 + rr + dh : 2 + rr + CHUNK + dh, 2 + dw : 2 + dw + W
                            ]
                            nc.tensor.matmul(
                                pcs[j], lhsT, rhs, start=(idx == 0), stop=(idx == 17)
                            )
                    for j, rr in enumerate(rrs):
                        ob = outp.tile([128, CHUNK, W], YDT, name="ob")
                        nc.scalar.activation(
                            ob,
                            pcs[j],
                            mybir.ActivationFunctionType.Relu,
                            bias=b_sb[:, oi : oi + 1],
                            scale=1.0,
                        )
                        nc.sync.dma_start(out=y[oi, :, rr : rr + CHUNK, :], in_=ob)

            def body(sm, first=False):
                # conv rows rr..rr+3 read sm rows rr..rr+7 (interior rr-2..rr+5):
                # a chunk is ready once smoothing covers row rr+5. Batch
                # flushes (>=3 chunks) so one weight pass serves several
                # matmul columns.
                pending = list(range(0, H, CHUNK))
                emitted = [0]

                def flush(upto):
                    ready = [rr for rr in pending if rr + 6 <= upto or upto >= H]
                    if not ready:
                        return
                    thresh = flush_min_first if emitted[0] == 0 else flush_min
                    if upto < H and len(ready) < thresh:
                        return
                    for rr in ready:
                        pending.remove(rr)
                    emitted[0] += len(ready)
                    conv_group(sm, ready)

                prev = None
                for si, (r0, nr) in enumerate(SLABS):
                    prev = smooth(
                        sm, r0, nr, first=first and si == 0, prev=prev, flush=flush,
                        s5_on_pe=pe_s5 and si < len(SLABS) - s5_dve_tail,
                    )
                    if prev is None:
                        flush(r0 + nr)
                if prev is not None:
                    merge(sm, prev)
                flush(H)
                assert not pending

            if loop is not None:
                assert loop % unroll == 0
                with tc.For_i(0, loop // unroll, 1):
                    for i in range(unroll):
                        body(sms[i % n_sm], first=False)
            else:
                for r in range(repeats):
                    body(sms[0], first=(r == 0))

    nc.compile()
    return nc


def _prep(inputs, fp16=False):
    NPDT = np.float16 if fp16 else ml_dtypes.bfloat16
    x = np.asarray(inputs["x"], np.float32)
    pm = np.asarray(inputs["perspective_map"], np.float32)
    co = np.asarray(inputs["sigma_coeffs"], np.float32)
    Wc = np.asarray(inputs["conv_w"], np.float32)
    bb = np.asarray(inputs["conv_b"], np.float32)

    # per-pixel coefficient planes (host): c_m = t^m / Z, replicated over partitions
    p = pm[:, 0]  # [B,H,W]
    sigma = co[0] * p**3 + co[1] * p**2 + co[2] * p + co[3]
    sigma = np.maximum(sigma, 0.5)
    t = np.exp(-1.0 / (2.0 * sigma * sigma))
    Z = 1 + 4 * t + 4 * t**2 + 4 * t**4 + 8 * t**5 + 4 * t**8
    cm = np.stack([(t**m) / Z for m in MS], axis=1).astype(NPDT)  # [B,6,H,W]
    cpl = np.ascontiguousarray(np.broadcast_to(cm[:, None], (B, 128, 6, H, W)))

    # zero-padded input: [B, 128(part), 2(ct), HP, WP]
    xpad = np.zeros((B, 128, 2, HP, WP), NPDT)
    xpad[:, :, :, 2 : H + 2, 2 : W + 2] = (
        x.astype(NPDT).reshape(B, 2, 128, H, W).transpose(0, 2, 1, 3, 4)
    )

    # conv weights: lhsT layout [ki, 128(i), q, oi, 128(o)]
    Wt = Wc.transpose(1, 0, 2, 3).astype(NPDT)  # [I, O, kh, kw]
    wts = np.empty((2, 128, 9, 2, 128), NPDT)
    for ki in range(2):
        for q in range(9):
            kh, kw = q // 3, q % 3
            for oi in range(2):
                wts[ki, :, q, oi, :] = Wt[
                    ki * 128 : (ki + 1) * 128, oi * 128 : (oi + 1) * 128, kh, kw
                ]
    wts = wts.reshape(2, 128, 9 * 2 * 128)
    bias_h = np.ascontiguousarray(bb.reshape(2, 128).T.astype(np.float32))  # [128, 2]
    ident = np.eye(128, dtype=NPDT)

    return [
        {"xp": xpad[b], "cpl": cpl[b], "wts": wts, "bias": bias_h, "ident": ident}
        for b in range(B)
    ]


def _get_nc(repeats=1, loop=None, **kw):
    key = ("nc", repeats, loop, tuple(sorted(kw.items())))
    if key not in _cache:
        _cache[key] = _build(repeats, loop, **kw)
    return _cache[key]


def run(inputs, trace=False, **kw):
    from concourse.bass_utils import run_bass_kernel_spmd

    nc = _get_nc()
    in_maps = _prep(inputs)
    res = run_bass_kernel_spmd(nc, in_maps, core_ids=list(range(B)), trace=trace, **kw)
    out = np.stack([r["y"].reshape(C, H, W) for r in res.results]).astype(np.float32)
    return out, res


def kernel(**inputs):
    out, _ = run(inputs)
    return out


# revision 38
# speedup vs baseline: 2.0374x; 1.0402x over previous
"""Trainium2 Bass kernel for BasicPGCBlock:
   per-pixel Gaussian smoothing (5x5, sigma = cubic(perspective)) -> dilated 3x3 conv (256->256) + bias + ReLU.

Sharding: data-parallel over batch, 1 image per NeuronCore (8 cores).

Math: the per-pixel 5x5 kernel w(u,v) = exp(-(u^2+v^2)/(2 s^2)) / Z factors through
t = exp(-1/(2 s^2)):  w(u,v) = t^(u^2+v^2) / Z, and u^2+v^2 in {0,1,2,4,5,8}.
So smoothed = sum_m c_m * S_m with c_m = t^m / Z (host-computed per-pixel planes,
replicated across partitions) and S_m = fixed 0/1 stencil sums of x built from
shifted adds.

Engine balance (HW-measured):
  PE:   dilated conv (18 accumulating matmuls per 4-row chunk per half-out) +
        S5 (4 taps) and S2 (2 taps) built via identity matmuls in PSUM.
  DVE:  P1/P2 horizontal sums, S1/S4/S8 vertical builds, the 6-term apply
        chain (bf16 2x mode, ~0.56 ns/elem).
  Act:  S5/S2 PSUM evacuation, conv bias+ReLU evacuation.
  Pool (gpsimd): UNUSED — measured ~10x slower than the cost model claims
        for strided/broadcast tensor ops on real TRN2; offloading any
        elementwise to it regressed wall time by ~70-150us.

All smoothing in bf16 (DVE 2x mode; fp16 measured ~4% slower end-to-end),
conv in bf16 with f32 PSUM accumulation. 8-row slabs pipeline smoothing
against the conv; conv chunks flush as soon as their sm rows settle.
"""

import sys

sys.path.insert(0, "/opt/trn_rl_repo")

import numpy as np
import ml_dtypes

B, C, H, W = 8, 256, 96, 96
HP, WP = H + 4, W + 4          # zero-padded by 2 on each side
CHUNK = 4                      # conv output rows per matmul (N = 4*96 = 384 <= 512)
OFFS = (-2, 0, 2)              # dilated conv offsets
MS = (0, 1, 2, 4, 5, 8)        # exponents of t present in the 5x5 kernel

_cache = {}


def _slabs(n=8):
    assert H % n == 0
    return tuple((r, n) for r in range(0, H, n))


def _build(repeats=1, loop=None, fp16=False, pool_ops=0, pe_s5=True,
           pe_s2=True, pe_s8=False, slabs=None, unroll=8, flush_min=2,
           flush_min_first=2, s5_dve_tail=0, pingpong=False, y_bf16=False):
    import concourse.mybir as mybir
    from concourse import bacc
    from concourse.tile import TileContext

    dt = mybir.dt
    DT = dt.float16 if fp16 else dt.bfloat16
    SLABS = slabs if slabs is not None else _slabs()
    nc = bacc.Bacc("TRN2", target_bir_lowering=False, debug=False)

    xp = nc.dram_tensor("xp", (128, 2, HP, WP), DT, kind="ExternalInput").ap()
    cpl = nc.dram_tensor("cpl", (128, 6, H, W), DT, kind="ExternalInput").ap()
    wts = nc.dram_tensor("wts", (2, 128, 9 * 2 * 128), DT, kind="ExternalInput").ap()
    bias = nc.dram_tensor("bias", (128, 2), dt.float32, kind="ExternalInput").ap()
    ident = nc.dram_tensor("ident", (128, 128), DT, kind="ExternalInput").ap()
    YDT = dt.bfloat16 if y_bf16 else dt.float32
    y = nc.dram_tensor("y", (2, 128, H, W), YDT, kind="ExternalOutput").ap()

    with TileContext(nc) as tc:
        with (
            tc.tile_pool(name="const", bufs=1) as constp,
            tc.tile_pool(name="smpool", bufs=1) as smpool,
            tc.tile_pool(name="io", bufs=2) as iop,
            tc.tile_pool(name="tmp", bufs=1) as tmp,
            tc.tile_pool(name="outp", bufs=6) as outp,
            tc.tile_pool(name="psum", bufs=8, space="PSUM") as psp,
        ):
            id_sb = constp.tile([128, 128], DT)
            nc.sync.dma_start(out=id_sb, in_=ident)
            w_sb = constp.tile([128, 2, 9 * 2 * 128], DT)
            b_sb = constp.tile([128, 2], dt.float32)

            def load_consts():
                # emitted after the first slab's input DMAs: conv weights are
                # not needed until well into the first slab
                nc.sync.dma_start(out=w_sb[:, 0], in_=wts[0])
                nc.sync.dma_start(out=w_sb[:, 1], in_=wts[1])
                nc.sync.dma_start(out=b_sb, in_=bias)

            if loop is not None:
                # looped timing variant: weights must be resident before the
                # hardware loop body runs
                load_consts()

            n_sm = min(unroll, 2) if (loop is not None and pingpong) else 1
            sms = []
            for i in range(n_sm):
                sm = smpool.tile([128, 2, HP, WP], DT, name=f"sm{i}")
                # zero only the 2-wide pad ring; the interior is fully
                # rewritten every pass
                nc.vector.memset(sm[:, :, 0:2, :], 0.0)
                nc.vector.memset(sm[:, :, HP - 2 : HP, :], 0.0)
                nc.vector.memset(sm[:, :, 2 : HP - 2, 0:2], 0.0)
                nc.vector.memset(sm[:, :, 2 : HP - 2, WP - 2 : WP], 0.0)
                sms.append(sm)

            def merge(sm, st):
                # deferred merge: combine slab st's DVE partial (acc) with the
                # Pool partial (m2+m8) one slab later, giving Pool a full slab
                # of slack before its outputs are consumed.
                r0, nr, acc, m2, m8 = st
                sm_out = sm[:, :, 2 + r0 : 2 + r0 + nr, 2 : W + 2]
                Bm = tmp.tile([128, 2, nr, W], DT, name="Bm", bufs=1)
                nc.vector.tensor_add(Bm, m2, m8)
                nc.vector.tensor_add(sm_out, acc, Bm)

            def smooth(sm, r0, nr, first=False, prev=None, flush=None,
                       s5_on_pe=None):
                s5_on_pe = pe_s5 if s5_on_pe is None else s5_on_pe
                xs = iop.tile([128, 2, nr + 4, WP], DT, name="xs")
                nc.sync.dma_start(out=xs, in_=xp[:, :, r0 : r0 + nr + 4, :])
                cp = iop.tile([128, 6, nr, W], DT, name="cp")
                nc.sync.dma_start(out=cp, in_=cpl[:, :, r0 : r0 + nr, :])
                if first:
                    load_consts()

                def cpm(m):
                    i = MS.index(m)
                    return cp[:, i : i + 1].to_broadcast([128, 2, nr, W])

                P0 = xs[:, :, :, 2 : W + 2]
                P1 = tmp.tile([128, 2, nr + 4, W], DT, name="P1", bufs=2)
                nc.vector.tensor_add(P1, xs[:, :, :, 1 : W + 1], xs[:, :, :, 3 : W + 3])
                P2 = tmp.tile([128, 2, nr + 4, W], DT, name="P2", bufs=2)
                nc.vector.tensor_add(P2, xs[:, :, :, 0:W], xs[:, :, :, 4 : W + 4])

                if prev is not None:
                    # merge the PREVIOUS slab now that Pool has had a slab of
                    # runway, then release its conv chunks
                    merge(sm, prev)
                    if flush is not None:
                        flush(prev[0] + prev[1])

                ctr = lambda P: P[:, :, 2 : nr + 2]
                u1 = lambda P: P[:, :, 1 : nr + 1]
                d1 = lambda P: P[:, :, 3 : nr + 3]
                u2 = lambda P: P[:, :, 0:nr]
                d2 = lambda P: P[:, :, 4 : nr + 4]

                # --- S2/S8 + their multiplies on Pool (gpsimd), S5 on PE ---
                # pool_ops: number of the ops [S2, S8, m2, m8] offloaded to
                # the Pool engine (rest stay on DVE).
                if pool_ops:
                    def eng(i):
                        return nc.gpsimd if i < pool_ops else nc.vector

                    S2 = tmp.tile([128, 2, nr, W], DT, name="S2", bufs=1)
                    eng(0).tensor_add(S2, u1(P1), d1(P1))
                    S8 = tmp.tile([128, 2, nr, W], DT, name="S8", bufs=1)
                    eng(1).tensor_add(S8, u2(P2), d2(P2))
                    m2 = tmp.tile([128, 2, nr, W], DT, name="m2", bufs=2)
                    eng(2).tensor_mul(m2, S2, cpm(2))
                    m8 = tmp.tile([128, 2, nr, W], DT, name="m8", bufs=2)
                    eng(3).tensor_mul(m8, S8, cpm(8))

                def pe_stencil(name, taps):
                    # k-tap stencil sum on TensorE via identity matmuls
                    # accumulated in PSUM, evacuated to SBUF by ScalarE.
                    out_t = tmp.tile([128, 2, nr, W], DT, name=name, bufs=2)
                    for ct in range(2):
                        for rk in range(nr // CHUNK):
                            rs = CHUNK * rk
                            pc5 = psp.tile(
                                [128, CHUNK, W], dt.float32, name="pc5",
                                bufs=3 if pe_s8 else 2,
                            )
                            for j, Pv in enumerate(taps):
                                nc.tensor.matmul(
                                    pc5,
                                    id_sb,
                                    Pv[:, ct, rs : rs + CHUNK, :],
                                    start=(j == 0),
                                    stop=(j == len(taps) - 1),
                                )
                            nc.scalar.activation(
                                out_t[:, ct, rs : rs + CHUNK, :],
                                pc5,
                                mybir.ActivationFunctionType.Copy,
                            )
                    return out_t

                if s5_on_pe:
                    # S5 = (P1[h-2]+P1[h+2]) + (P2[h-1]+P2[h+1])
                    S5 = pe_stencil("S5", (u2(P1), d2(P1), u1(P2), d1(P2)))
                if pe_s2 and s5_on_pe:
                    S2 = pe_stencil("S2p", (u1(P1), d1(P1)))
                if pe_s8 and s5_on_pe:
                    S8 = pe_stencil("S8p", (u2(P2), d2(P2)))

                # --- S1/S4 builds + apply chain on DVE ---
                Q = tmp.tile([128, 2, nr, W], DT, name="Q", bufs=2)
                nc.vector.tensor_add(Q, u1(P0), d1(P0))
                S1 = tmp.tile([128, 2, nr, W], DT, name="S1", bufs=2)
                nc.vector.tensor_add(S1, Q, ctr(P1))
                Q2 = tmp.tile([128, 2, nr, W], DT, name="Q", bufs=2)
                nc.vector.tensor_add(Q2, u2(P0), d2(P0))
                S4 = tmp.tile([128, 2, nr, W], DT, name="S4", bufs=2)
                nc.vector.tensor_add(S4, Q2, ctr(P2))

                if not pool_ops:
                    if not (pe_s2 and s5_on_pe):
                        S2 = tmp.tile([128, 2, nr, W], DT, name="S2v", bufs=2)
                        nc.vector.tensor_add(S2, u1(P1), d1(P1))
                    if not (pe_s8 and s5_on_pe):
                        S8 = tmp.tile([128, 2, nr, W], DT, name="S8v", bufs=2)
                        nc.vector.tensor_add(S8, u2(P2), d2(P2))
                if not s5_on_pe:
                    S5 = tmp.tile([128, 2, nr, W], DT, name="S5v", bufs=2)
                    nc.vector.tensor_add(S5, u2(P1), d2(P1))
                    Qb = tmp.tile([128, 2, nr, W], DT, name="Qb", bufs=2)
                    nc.vector.tensor_add(Qb, u1(P2), d1(P2))
                    nc.vector.tensor_add(S5, S5, Qb)

                acc = tmp.tile([128, 2, nr, W], DT, name="acc", bufs=2)
                nc.vector.tensor_mul(acc, ctr(P0), cpm(0))
                t = tmp.tile([128, 2, nr, W], DT, name="t", bufs=3)
                nc.vector.tensor_mul(t, S1, cpm(1))
                nc.vector.tensor_add(acc, acc, t)
                t4 = tmp.tile([128, 2, nr, W], DT, name="t", bufs=3)
                nc.vector.tensor_mul(t4, S4, cpm(4))
                nc.vector.tensor_add(acc, acc, t4)
                t5 = tmp.tile([128, 2, nr, W], DT, name="t", bufs=3)
                nc.vector.tensor_mul(t5, S5, cpm(5))
                nc.vector.tensor_add(acc, acc, t5)

                if pool_ops:
                    return (r0, nr, acc, m2, m8)
                sm_out = sm[:, :, 2 + r0 : 2 + r0 + nr, 2 : W + 2]
                t2 = tmp.tile([128, 2, nr, W], DT, name="t", bufs=3)
                nc.vector.tensor_mul(t2, S2, cpm(2))
                nc.vector.tensor_add(acc, acc, t2)
                t8 = tmp.tile([128, 2, nr, W], DT, name="t", bufs=3)
                nc.vector.tensor_mul(t8, S8, cpm(8))
                nc.vector.tensor_add(sm_out, acc, t8)
                return None

            def conv_group(sm, rrs):
                # rrs: output-row starts whose sm dependencies are met; one
                # weight load serves len(rrs) matmuls.
                for oi in range(2):
                    pcs = [
                        psp.tile([128, CHUNK, W], dt.float32, name="pc",
                                 bufs=5 if pe_s8 else 6)
                        for _ in rrs
                    ]
                    for idx in range(18):
                        ki, q = idx // 9, idx % 9
                        dh, dw = OFFS[q // 3], OFFS[q % 3]
                        lhsT = w_sb[:, ki, (q * 2 + oi) * 128 : (q * 2 + oi + 1) * 128]
                        for j, rr in enumerate(rrs):
                            rhs = sm[
                                :, ki, 2 + rr + dh : 2 + rr + CHUNK + dh, 2 + dw : 2 + dw + W
                            ]
                            nc.tensor.matmul(
                                pcs[j], lhsT, rhs, start=(idx == 0), stop=(idx == 17)
                            )
                    for j, rr in enumerate(rrs):
                        ob = outp.tile([128, CHUNK, W], YDT, name="ob")
                        nc.scalar.activation(
                            ob,
                            pcs[j],
                            mybir.ActivationFunctionType.Relu,
                            bias=b_sb[:, oi : oi + 1],
                            scale=1.0,
                        )
                        nc.sync.dma_start(out=y[oi, :, rr : rr + CHUNK, :], in_=ob)

            def body(sm, first=False):
                # conv rows rr..rr+3 read sm rows rr..rr+7 (interior rr-2..rr+5):
                # a chunk is ready once smoothing covers row rr+5. Batch
                # flushes (>=3 chunks) so one weight pass serves several
                # matmul columns.
                pending = list(range(0, H, CHUNK))
                emitted = [0]

                def flush(upto):
                    ready = [rr for rr in pending if rr + 6 <= upto or upto >= H]
                    if not ready:
                        return
                    thresh = flush_min_first if emitted[0] == 0 else flush_min
                    if upto < H and len(ready) < thresh:
                        return
                    for rr in ready:
                        pending.remove(rr)
                    emitted[0] += len(ready)
                    conv_group(sm, ready)

                prev = None
                for si, (r0, nr) in enumerate(SLABS):
                    prev = smooth(
                        sm, r0, nr, first=first and si == 0, prev=prev, flush=flush,
                        s5_on_pe=pe_s5 and si < len(SLABS) - s5_dve_tail,
                    )
                    if prev is None:
                        flush(r0 + nr)
                if prev is not None:
                    merge(sm, prev)
                flush(H)
                assert not pending

            if loop is not None:
                assert loop % unroll == 0
                with tc.For_i(0, loop // unroll, 1):
                    for i in range(unroll):
                        body(sms[i % n_sm], first=False)
            else:
                for r in range(repeats):
                    body(sms[0], first=(r == 0))

    nc.compile()
    return nc


def _prep(inputs, fp16=False):
    NPDT = np.float16 if fp16 else ml_dtypes.bfloat16
    x = np.asarray(inputs["x"], np.float32)
    pm = np.asarray(inputs["perspective_map"], np.float32)
    co = np.asarray(inputs["sigma_coeffs"], np.float32)
    Wc = np.asarray(inputs["conv_w"], np.float32)
    bb = np.asarray(inputs["conv_b"], np.float32)

    # per-pixel coefficient planes (host): c_m = t^m / Z, replicated over partitions
    p = pm[:, 0]  # [B,H,W]
    sigma = co[0] * p**3 + co[1] * p**2 + co[2] * p + co[3]
    sigma = np.maximum(sigma, 0.5)
    t = np.exp(-1.0 / (2.0 * sigma * sigma))
    Z = 1 + 4 * t + 4 * t**2 + 4 * t**4 + 8 * t**5 + 4 * t**8
    cm = np.stack([(t**m) / Z for m in MS], axis=1).astype(NPDT)  # [B,6,H,W]
    cpl = np.ascontiguousarray(np.broadcast_to(cm[:, None], (B, 128, 6, H, W)))

    # zero-padded input: [B, 128(part), 2(ct), HP, WP]
    xpad = np.zeros((B, 128, 2, HP, WP), NPDT)
    xpad[:, :, :, 2 : H + 2, 2 : W + 2] = (
        x.astype(NPDT).reshape(B, 2, 128, H, W).transpose(0, 2, 1, 3, 4)
    )

    # conv weights: lhsT layout [ki, 128(i), q, oi, 128(o)]
    Wt = Wc.transpose(1, 0, 2, 3).astype(NPDT)  # [I, O, kh, kw]
    wts = np.empty((2, 128, 9, 2, 128), NPDT)
    for ki in range(2):
        for q in range(9):
            kh, kw = q // 3, q % 3
            for oi in range(2):
                wts[ki, :, q, oi, :] = Wt[
                    ki * 128 : (ki + 1) * 128, oi * 128 : (oi + 1) * 128, kh, kw
                ]
    wts = wts.reshape(2, 128, 9 * 2 * 128)
    bias_h = np.ascontiguousarray(bb.reshape(2, 128).T.astype(np.float32))  # [128, 2]
    ident = np.eye(128, dtype=NPDT)

    return [
        {"xp": xpad[b], "cpl": cpl[b], "wts": wts, "bias": bias_h, "ident": ident}
        for b in range(B)
    ]


def _get_nc(repeats=1, loop=None, **kw):
    key = ("nc", repeats, loop, tuple(sorted(kw.items())))
    if key not in _cache:
        _cache[key] = _build(repeats, loop, **kw)
    return _cache[key]


def run(inputs, trace=False, **kw):
    from concourse.bass_utils import run_bass_kernel_spmd

    nc = _get_nc()
    in_maps = _prep(inputs)
    res = run_bass_kernel_spmd(nc, in_maps, core_ids=list(range(B)), trace=trace, **kw)
    out = np.stack([r["y"].reshape(C, H, W) for r in res.results]).astype(np.float32)
    return out, res


def kernel(**inputs):
    out, _ = run(inputs)
    return out
